# revision 20
# baseline (speedup 1.0000x reference)
"""GCN (GraphConv norm='both' -> ReLU -> SAGEConv mean) on 8 Trainium2 NeuronCores.

Contract: kernel(**inputs) takes the FULL inputs from setup_inputs() and
returns the FULL [N, OUT] output.

Sharding strategy (graph/data parallel, per the problem's sharding hint):
  - Nodes are partitioned contiguously across the 8 cores (12500 each).
  - Edges are partitioned by the owner of their *dst* node; each core's
    edges are bucketed by (128-node dst window, 25000-row src range) --
    the range split because dma_gather indices are int16 -- and padded to
    128-edge chunks. Chunk structure is the max over cores so the SPMD
    program is identical on all 8.
  - Weights are replicated.
  - Phase 1 (per core): dma_gather x[src] rows from HBM (4 SWDGE queues),
    segment-sum via one-hot matmuls on the TensorEngine into PSUM (the
    edge weight s_out[src]*s_in[dst] folded into the one-hot values,
    which are built batched per window on the VectorEngine), then
    hT = relu(W1.T @ aggT + b1) kept SBUF-resident, and z = h @ W_neigh
    written to a local z shard.
  - z shards are AllGathered across the 8 cores (the halo exchange -- on
    this random graph the halo is ~the whole graph, and exchanging
    z = h @ W_neigh (64 wide) instead of h (128 wide) halves the traffic
    since (segsum h) @ W_neigh == segsum (h @ W_neigh)).
  - Phase 2 (per core): dma_gather z[src] rows, segment-sum with 0/1
    one-hots, scale by 1/deg_in per dst row (partition-wise), add
    h @ W_self + b2, write the core's [12500, 64] output shard.
  - Host concatenates the 8 shards.

Host-side prep is integer graph restructuring (edge partition / sort /
pad / degree counts) plus the per-edge phase-1 normalization weights
derived from the degree histograms; all O(N*F) / O(E*F) floating point
work (gathers, segment sums, matmuls, bias, relu, 1/deg scaling) runs on
the NeuronCores.
"""

import os
import sys
from contextlib import ExitStack

import numpy as np

for _p in ("/opt/trn_rl_repo", "/opt/pypackages"):
    if _p not in sys.path:
        sys.path.append(_p)

import concourse.bacc as bacc
import concourse.bass as bass
import concourse.mybir as mybir
import concourse.tile as tile
from concourse.bass_utils import run_bass_kernel_spmd

F32 = mybir.dt.float32
I16 = mybir.dt.int16
AOT = mybir.AluOpType
AFT = mybir.ActivationFunctionType

N_CORES = 8
WIN = 128
MAXRANGE = 32768  # dma_gather idx is int16
GROUP1 = 4  # windows per gather slab group
SUBCHUNKS = 8  # chunks per dma_gather instruction (1024 idxs = 64 desc/engine)
NQUEUES = 4


def _install_ntff_hook_shim():
    """The agent image's antenv lacks axon_hooks; provide it so trace=True
    can capture NTFF profiles through libaxon (same hook trn_boot would
    register). No-op if the real module exists or libaxon lacks support."""
    try:
        from antenv import axon_hooks  # noqa: F401
        return
    except ImportError:
        pass
    try:
        import types

        import antenv
        from trn_agent_boot.trn_boot import _ntff_profile_via_ctypes

        mod = types.ModuleType("antenv.axon_hooks")
        mod._hook = _ntff_profile_via_ctypes("/opt/axon/libaxon_pjrt.so")

        def get_axon_ntff_profile_hook():
            return mod._hook

        def set_axon_ntff_profile_hook(h):
            mod._hook = h

        mod.get_axon_ntff_profile_hook = get_axon_ntff_profile_hook
        mod.set_axon_ntff_profile_hook = set_axon_ntff_profile_hook
        sys.modules["antenv.axon_hooks"] = mod
        antenv.axon_hooks = mod
    except Exception:
        pass


_install_ntff_hook_shim()


# ---------------------------------------------------------------------------
# Host-side graph prep
# ---------------------------------------------------------------------------

class Prep:
    pass


def prepare(src, dst, n_nodes, n_cores=N_CORES):
    src = np.asarray(src).astype(np.int64)
    dst = np.asarray(dst).astype(np.int64)
    P = n_nodes // n_cores
    assert P * n_cores == n_nodes
    NW = (P + WIN - 1) // WIN
    rows_last = P - WIN * (NW - 1)
    RSZ = MAXRANGE if n_nodes > MAXRANGE else -(-n_nodes // 4)
    NRANGES = -(-n_nodes // RSZ)
    assert RSZ <= MAXRANGE

    deg_out = np.bincount(src, minlength=n_nodes).astype(np.float32)
    deg_in = np.bincount(dst, minlength=n_nodes).astype(np.float32)
    s_out = 1.0 / np.sqrt(np.maximum(deg_out, 1.0))
    s_in = 1.0 / np.sqrt(np.maximum(deg_in, 1.0))
    sw1_all = (s_out[src] * s_in[dst]).astype(np.float32)

    # per-dst-node 1/max(deg_in,1) in [partition, window] layout per core
    invd = (1.0 / np.maximum(deg_in, 1.0)).astype(np.float32)

    owner = dst // P
    ldst = dst - owner * P
    wrow = ldst // WIN
    code = (ldst % WIN).astype(np.float32)
    rng_of = src // RSZ

    counts = np.zeros((n_cores, NW, NRANGES), np.int64)
    np.add.at(counts, (owner, wrow, rng_of), 1)
    cwr = (counts.max(axis=0) + WIN - 1) // WIN  # [NW, NRANGES]
    empty = cwr.sum(axis=1) == 0
    cwr[empty, 0] = 1
    n_w = cwr.sum(axis=1)  # chunks per window

    groups1 = [(g0, min(g0 + GROUP1, NW)) for g0 in range(0, NW, GROUP1)]

    # gather-order (group -> range -> window -> chunk) cell starts
    cell_start = np.zeros((NW, NRANGES), np.int64)
    slab_meta = []
    c = 0
    for g0, g1 in groups1:
        metas = []
        for r in range(NRANGES):
            s = c
            for w in range(g0, g1):
                cell_start[w, r] = c
                c += int(cwr[w, r])
            metas.append((s, c - s))
        slab_meta.append(metas)
    C = c

    # sub-gather blocks in issue order, aligned to (window, range) cells so
    # pad indices (-1) are strictly trailing within each block:
    # (group, range, chunk_off_in_slab, nb, global_chunk_start)
    gathers = []
    for g, (g0, g1) in enumerate(groups1):
        for r in range(NRANGES):
            s, n = slab_meta[g][r]
            for w in range(g0, g1):
                cell0 = int(cell_start[w, r])
                for i in range(0, int(cwr[w, r]), SUBCHUNKS):
                    nb = min(SUBCHUNKS, int(cwr[w, r]) - i)
                    gathers.append((g, r, cell0 - s + i, nb, cell0 + i))
    NG = len(gathers)

    # window-major chunk columns (for the batched one-hot code/weight arrays)
    wc0 = np.zeros(NW, np.int64)
    wc0[1:] = np.cumsum(n_w)[:-1]

    # per window: ordered (range, gather_chunk_id) matching window-major order
    window_chunks = []
    for w in range(NW):
        lst = []
        for r in range(NRANGES):
            for j in range(int(cwr[w, r])):
                lst.append((r, int(cell_start[w, r]) + j))
        window_chunks.append(lst)

    per_core = []
    for k in range(n_cores):
        m = owner == k
        e_src = src[m]
        key = wrow[m] * NRANGES + rng_of[m]
        order = np.argsort(key, kind="stable")
        e_src = e_src[order]
        key = key[order]
        e_code = code[m][order]
        e_sw1 = sw1_all[m][order]
        bounds = np.searchsorted(key, np.arange(NW * NRANGES + 1))

        # gather-order idx array (pads = 0: they gather row 0 harmlessly and
        # their zero one-hot rows contribute nothing); window-major arrays
        A_idx = np.full(C * WIN, 0, np.int16)
        A_real = np.zeros(C * WIN, bool)
        W_code = np.full(C * WIN, 255.0, np.float32)
        W_sw1 = np.zeros(C * WIN, np.float32)
        for w in range(NW):
            woff = 0
            for r in range(NRANGES):
                a, b = bounds[w * NRANGES + r], bounds[w * NRANGES + r + 1]
                n = b - a
                gbase = int(cell_start[w, r]) * WIN
                wbase = (int(wc0[w]) + woff) * WIN
                woff += int(cwr[w, r])
                if n == 0:
                    continue
                A_idx[gbase : gbase + n] = (e_src[a:b] - r * RSZ).astype(np.int16)
                A_real[gbase : gbase + n] = True
                W_code[wbase : wbase + n] = e_code[a:b]
                W_sw1[wbase : wbase + n] = e_sw1[a:b]

        idx_cols = []
        gcnt = np.zeros(NG, np.int32)
        for gi, (g, r, i, nb, cs) in enumerate(gathers):
            seg = A_idx[cs * WIN : (cs + nb) * WIN].copy()
            nreal = int(A_real[cs * WIN : (cs + nb) * WIN].sum())
            if nreal == 0:
                seg[0] = 0  # keep at least one valid descriptor
                nreal = 1
            gcnt[gi] = nreal
            idx_cols.append(np.tile(seg.reshape(-1, 16).T, (8, 1)))
        eidx = np.ascontiguousarray(np.concatenate(idx_cols, axis=1))
        assert eidx.shape == (128, C * 8)

        def tr(a):
            return np.ascontiguousarray(a.reshape(C, WIN).T)

        # invd in [partition, window] layout for this core's nodes
        nodes = np.arange(P) + k * P
        iv = np.zeros(NW * WIN, np.float32)
        iv[:P] = invd[nodes]
        invd_pw = np.ascontiguousarray(iv.reshape(NW, WIN).T)

        per_core.append(
            dict(eidx=eidx, ecode=tr(W_code), esw1=tr(W_sw1), invd=invd_pw,
                 gcnt=gcnt.reshape(1, NG))
        )

    p = Prep()
    p.P, p.NW, p.rows_last, p.C, p.RSZ = P, NW, rows_last, C, RSZ
    p.nranges = NRANGES
    p.cwr = cwr
    p.n_w = n_w
    p.wc0 = wc0
    p.groups1 = groups1
    p.slab_meta = slab_meta
    p.gathers = gathers
    p.NG = NG
    p.window_chunks = window_chunks
    p.per_core = per_core
    p.n_nodes = n_nodes
    p.n_cores = n_cores
    return p


# ---------------------------------------------------------------------------
# Bass/Tile kernel builder
# ---------------------------------------------------------------------------

def build_gcn(p, F, H, O, gather_bufs=2):
    NW, C, P, RSZ = p.NW, p.C, p.P, p.RSZ
    NRANGES = p.nranges
    max_nw = int(p.n_w.max())
    max_slab = [
        max((p.slab_meta[g][r][1] for g in range(len(p.groups1))), default=0)
        for r in range(NRANGES)
    ]
    # gathers grouped by (g, r): list of (gi, chunk_off_in_slab, nb, chunk_start)
    by_slab = {}
    for gi, (g, r, i, nb, cs) in enumerate(p.gathers):
        by_slab.setdefault((g, r), []).append((gi, i, nb, cs))
    group_c0 = [p.slab_meta[g][0][0] for g in range(len(p.groups1))]
    group_c1 = group_c0[1:] + [C]

    nc = bacc.Bacc(
        "TRN2", debug=False, enable_asserts=False, num_devices=p.n_cores,
        num_swdge_queues=NQUEUES,
    )

    x_d = nc.dram_tensor("x", [p.n_nodes, F], F32, kind="ExternalInput").ap()
    W1_d = nc.dram_tensor("W1", [F, H], F32, kind="ExternalInput").ap()
    b1_d = nc.dram_tensor("b1", [H, 1], F32, kind="ExternalInput").ap()
    Ws_d = nc.dram_tensor("W_self", [H, O], F32, kind="ExternalInput").ap()
    Wn_d = nc.dram_tensor("W_neigh", [H, O], F32, kind="ExternalInput").ap()
    b2_d = nc.dram_tensor("b2", [1, O], F32, kind="ExternalInput").ap()
    eidx_d = nc.dram_tensor("eidx", [WIN, C * 8], I16, kind="ExternalInput").ap()
    ecode_d = nc.dram_tensor("ecode", [WIN, C], F32, kind="ExternalInput").ap()
    esw1_d = nc.dram_tensor("esw1", [WIN, C], F32, kind="ExternalInput").ap()
    invd_d = nc.dram_tensor("invd", [WIN, NW], F32, kind="ExternalInput").ap()
    gcnt_d = nc.dram_tensor("gcnt", [1, p.NG], mybir.dt.int32, kind="ExternalInput").ap()
    out_d = nc.dram_tensor("out", [P, O], F32, kind="ExternalOutput").ap()

    qn = [0]

    def next_q():
        q = qn[0]
        qn[0] = (q + 1) % NQUEUES
        return q

    with tile.TileContext(nc, num_cores=p.n_cores) as tc, ExitStack() as ctx:
        const = ctx.enter_context(tc.tile_pool(name="const", bufs=1))
        dram = ctx.enter_context(tc.tile_pool(name="dram", bufs=1, space="DRAM"))

        W1s = const.tile([F, H], F32)
        nc.sync.dma_start(W1s[:], W1_d)
        Wss = const.tile([H, O], F32)
        nc.sync.dma_start(Wss[:], Ws_d)
        Wns = const.tile([H, O], F32)
        nc.sync.dma_start(Wns[:], Wn_d)
        b1s = const.tile([H, 1], F32)
        nc.sync.dma_start(b1s[:], b1_d)
        b2s = const.tile([1, O], F32)
        nc.sync.dma_start(b2s[:], b2_d)
        ecode_s = const.tile([WIN, C], F32)
        nc.sync.dma_start(ecode_s[:], ecode_d)
        esw1_s = const.tile([WIN, C], F32)
        nc.sync.dma_start(esw1_s[:], esw1_d)
        invd_s = const.tile([WIN, NW], F32)
        nc.sync.dma_start(invd_s[:], invd_d)
        gcnt_s = const.tile([1, p.NG], mybir.dt.int32)
        nc.sync.dma_start(gcnt_s[:], gcnt_d)

        ones1 = const.tile([1, WIN], F32)
        nc.vector.memset(ones1[:], 1.0)
        iota = const.tile([WIN, WIN], F32)
        nc.gpsimd.iota(
            iota[:],
            pattern=[[1, WIN]],
            base=0,
            channel_multiplier=0,
            allow_small_or_imprecise_dtypes=True,
        )

        hT = const.tile([H, NW * WIN], F32)

        zshard = dram.tile([P, O], F32)
        zfull = dram.tile([p.n_nodes, O], F32, addr_space="Shared")

        def load_group_idx(pool, g):
            c0, c1 = group_c0[g], group_c1[g]
            t = pool.tile([WIN, (c1 - c0) * 8], I16, tag="gidx")
            nc.sync.dma_start(t[:], eidx_d[:, c0 * 8 : c1 * 8])
            return t, c0

        def gather_slab(pool, g, r, src_ap, elem, tag, gidx, gidx_c0, memset):
            s, n = p.slab_meta[g][r]
            if n == 0:
                return None, s
            t = pool.tile([WIN, max_slab[r], elem], F32, tag=tag)
            if memset:
                nc.vector.memset(t[:], 0.0)
            r0 = r * RSZ
            r1 = min(r0 + RSZ, p.n_nodes)
            for gi, i, nb, cs in by_slab[(g, r)]:
                nc.gpsimd.dma_gather(
                    out_ap=t[:, i : i + nb, :],
                    in_ap=src_ap[r0:r1, :],
                    idxs_ap=gidx[:, (cs - gidx_c0) * 8 : (cs - gidx_c0 + nb) * 8],
                    num_idxs=nb * WIN,
                    num_idxs_reg=nb * WIN,
                    elem_size=elem,
                    queue_num=next_q(),
                )
            return t, s

        def build_eq(pool, w, weighted):
            """Batched one-hot for window w: [WIN, n_w, WIN]."""
            n = int(p.n_w[w])
            c0 = int(p.wc0[w])
            eq = pool.tile([WIN, max_nw, WIN], F32, tag="eq")
            nc.vector.tensor_tensor(
                out=eq[:, :n, :],
                in0=ecode_s[:, c0 : c0 + n].to_broadcast([WIN, n, WIN]),
                in1=iota[:].rearrange("p f -> p () f").to_broadcast([WIN, n, WIN]),
                op=AOT.is_equal,
            )
            if weighted:
                nc.vector.tensor_tensor(
                    out=eq[:, :n, :],
                    in0=eq[:, :n, :],
                    in1=esw1_s[:, c0 : c0 + n].to_broadcast([WIN, n, WIN]),
                    op=AOT.mult,
                )
            return eq

        # ---------------- phase 1 ----------------
        groups1 = p.groups1
        with (
            tc.tile_pool(name="xg", bufs=gather_bufs) as xgp,
            tc.tile_pool(name="gidx1", bufs=2) as gixp,
            tc.tile_pool(name="oh1", bufs=2) as ohp,
            tc.tile_pool(name="aggn", bufs=2) as aggp,
            tc.tile_pool(name="zt", bufs=2) as ztp,
            tc.tile_pool(name="psA", bufs=2, space="PSUM") as psA,
            tc.tile_pool(name="psH", bufs=2, space="PSUM") as psH,
            tc.tile_pool(name="psZ", bufs=2, space="PSUM") as psZ,
        ):
            for g, (g0, g1) in enumerate(groups1):
                gidx, gidx_c0 = load_group_idx(gixp, g)
                slabs = {}
                for r in range(NRANGES):
                    t, s = gather_slab(
                        xgp, g, r, x_d, F, f"xg{r}", gidx, gidx_c0,
                        memset=g < gather_bufs,
                    )
                    if t is not None:
                        slabs[r] = (t, s)

                for w in range(g0, g1):
                    rows = p.rows_last if w == NW - 1 else WIN
                    wsl = slice(w * WIN, (w + 1) * WIN)
                    chunks = p.window_chunks[w]

                    eq = build_eq(ohp, w, weighted=True)
                    agg = psA.tile([F, WIN], F32, tag="agg")
                    for jj, (r, gid) in enumerate(chunks):
                        t, s = slabs[r]
                        nc.tensor.matmul(
                            out=agg[:],
                            lhsT=t[:, gid - s, :],
                            rhs=eq[:, jj, :],
                            start=(jj == 0),
                            stop=(jj == len(chunks) - 1),
                        )

                    aggn = aggp.tile([F, WIN], F32, tag="aggn")
                    nc.scalar.activation(aggn[:], agg[:], AFT.Copy)

                    hpre = psH.tile([H, WIN], F32, tag="hpre")
                    nc.tensor.matmul(
                        out=hpre[:], lhsT=W1s[:], rhs=aggn[:], start=True, stop=True
                    )
                    nc.scalar.activation(hT[:, wsl], hpre[:], AFT.Relu, bias=b1s[:])

                    zp = psZ.tile([WIN, O], F32, tag="zp")
                    nc.tensor.matmul(
                        out=zp[:], lhsT=hT[:, wsl], rhs=Wns[:], start=True, stop=True
                    )
                    zt = ztp.tile([WIN, O], F32, tag="zt")
                    nc.vector.tensor_copy(zt[:], zp[:])
                    nc.sync.dma_start(
                        zshard[w * WIN : w * WIN + rows, :], zt[:rows, :]
                    )

        # ---------------- halo exchange ----------------
        nc.gpsimd.collective_compute(
            "AllGather",
            AOT.bypass,
            replica_groups=[list(range(p.n_cores))],
            ins=[zshard.opt()],
            outs=[zfull.opt()],
        )

        # ---------------- phase 2 ----------------
        with (
            tc.tile_pool(name="zg", bufs=gather_bufs) as zgp,
            tc.tile_pool(name="gidx2", bufs=2) as gixp2,
            tc.tile_pool(name="oh2", bufs=2) as ohp2,
            tc.tile_pool(name="nm", bufs=2) as nmp,
            tc.tile_pool(name="ot", bufs=2) as otp,
            tc.tile_pool(name="psN", bufs=2, space="PSUM") as psN,
            tc.tile_pool(name="psS", bufs=2, space="PSUM") as psS,
        ):
            for g, (g0, g1) in enumerate(groups1):
                gidx, gidx_c0 = load_group_idx(gixp2, g)
                slabs = {}
                for r in range(NRANGES):
                    t, s = gather_slab(
                        zgp, g, r, zfull.opt(), O, f"zg{r}", gidx, gidx_c0,
                        memset=g < gather_bufs,
                    )
                    if t is not None:
                        slabs[r] = (t, s)

                for w in range(g0, g1):
                    rows = p.rows_last if w == NW - 1 else WIN
                    wsl = slice(w * WIN, (w + 1) * WIN)
                    chunks = p.window_chunks[w]

                    eq = build_eq(ohp2, w, weighted=False)
                    nm = psN.tile([WIN, O], F32, tag="nm")
                    for jj, (r, gid) in enumerate(chunks):
                        t, s = slabs[r]
                        nc.tensor.matmul(
                            out=nm[:],
                            lhsT=eq[:, jj, :],
                            rhs=t[:, gid - s, :],
                            start=(jj == 0),
                            stop=(jj == len(chunks) - 1),
                        )

                    sb = psS.tile([WIN, O], F32, tag="sb")
                    nc.tensor.matmul(
                        out=sb[:], lhsT=ones1[:], rhs=b2s[:], start=True, stop=False
                    )
                    nc.tensor.matmul(
                        out=sb[:], lhsT=hT[:, wsl], rhs=Wss[:], start=False, stop=True
                    )

                    # nm * invd[dst] (partition-wise) then + (h@Ws + b2)
                    nms = nmp.tile([WIN, O], F32, tag="nms")
                    nc.vector.tensor_scalar(
                        out=nms[:], in0=nm[:], scalar1=invd_s[:, w : w + 1],
                        scalar2=None, op0=AOT.mult,
                    )
                    outt = otp.tile([WIN, O], F32, tag="outt")
                    nc.vector.tensor_tensor(outt[:], nms[:], sb[:], op=AOT.add)
                    nc.sync.dma_start(
                        out_d[w * WIN : w * WIN + rows, :], outt[:rows, :]
                    )

    nc.compile()
    return nc


# ---------------------------------------------------------------------------
# Entry point
# ---------------------------------------------------------------------------

_CACHE = {}


def _get_compiled(p, F, H, O):
    key = (p.n_nodes, p.n_cores, p.C, tuple(map(tuple, p.cwr.tolist())), F, H, O)
    if key not in _CACHE:
        import time as _time

        t0 = _time.time()
        _CACHE[key] = build_gcn(p, F, H, O)
        if os.environ.get("GCN_VERBOSE"):
            print(f"[gcn] build+bass-compile: {_time.time() - t0:.1f}s", flush=True)
    return _CACHE[key]


def make_in_maps(p, inputs):
    H = np.asarray(inputs["W1"]).shape[1]
    O = np.asarray(inputs["W_self"]).shape[1]
    base = dict(
        x=np.ascontiguousarray(np.asarray(inputs["x"], np.float32)),
        W1=np.ascontiguousarray(np.asarray(inputs["W1"], np.float32)),
        b1=np.ascontiguousarray(np.asarray(inputs["b1"], np.float32).reshape(H, 1)),
        W_self=np.ascontiguousarray(np.asarray(inputs["W_self"], np.float32)),
        W_neigh=np.ascontiguousarray(np.asarray(inputs["W_neigh"], np.float32)),
        b2=np.ascontiguousarray(np.asarray(inputs["b2"], np.float32).reshape(1, O)),
    )
    in_maps = []
    for k in range(p.n_cores):
        m = dict(base)
        m.update(p.per_core[k])
        in_maps.append(m)
    return in_maps


def kernel(**inputs):
    x = np.asarray(inputs["x"])
    src = np.asarray(inputs["src"])
    dst = np.asarray(inputs["dst"])
    n_nodes, F = x.shape
    H = np.asarray(inputs["W1"]).shape[1]
    O = np.asarray(inputs["W_self"]).shape[1]

    p = prepare(src, dst, n_nodes)
    nc = _get_compiled(p, F, H, O)
    in_maps = make_in_maps(p, inputs)
    res = run_bass_kernel_spmd(
        nc, in_maps, core_ids=list(range(p.n_cores)),
        trace=bool(int(os.environ.get("GCN_TRACE", "0"))),
    )
    if os.environ.get("GCN_RESULT_HOOK"):
        _CACHE["last_results"] = res
    out = np.concatenate([r["out"] for r in res.results], axis=0)
    return out.astype(np.float32)
